# revision 50
# baseline (speedup 1.0000x reference)
"""Trainium2 Bass kernel for nn_ExtractPatchesPositionLayer.

Reference semantics: per image b, out[r, c] = bilinear sample of the
(522,522,1) padded object at (r + 5 + py_b, c + 5 + px_b), zero fill
outside -> (512,512,1). Per image the shift is constant, so floor/frac
give an integer window start (A,B) plus two 1-D blend weight pairs
(1-wx,wx) and (1-wy,wy).

Host prep extracts each image's (513, 514) window (integer part of the
shift) into a packed, aligned layout, so the kernel sees fixed geometry:
only the blend weights vary per image, as per-partition scalars.

Layout: images are processed in PAIRS. Each SBUF partition holds EIGHT
consecutive window rows (8p..8p+7 plus a spill row 8p+8) as one
contiguous 64B-aligned ~9.3KB DRAM run; partitions 0:64 hold image 2t,
64:128 image 2t+1. Per pair: ONE dynamic input DMA (128 packets) and ONE
output store (128 x 8KB packets). All DMAs use a register-plus-immediate
(dynamic) offset: dynamic DMAs fan descriptors across all 16 DMA
engines, while static-offset DMAs serialize on one engine (the original
kernel's bottleneck: every output packet on engine 64 -> 1.42 ms).

Compute is pure elementwise fp16 (no PE/PSUM), two-stage blend with
q = 0..8 row-slices:
    vv[:, q, j] = (1-wy)*x[:, q, j] + wy*x[:, q+1, j]     q = 0..7
    v[:, q, j]  = (1-wx)*vv[:, q, j] + wx*vv[:, q, j+1]
via DVE tensor_scalar (4x fp16 mode) / tensor_tensor (2x) with the two
odd muls on ACT; the vertical q+1 neighbor lives in the SAME partition,
so there is no cross-partition data movement at all.

Sharding: batch 256 -> 32 images x 8 cores, embarrassingly parallel.
"""

from dataclasses import dataclass

import numpy as np

import concourse.bacc as bacc
import concourse.bass as bass
import concourse.mybir as mybir
import concourse.tile as tile
from concourse.bass_utils import run_bass_kernel_spmd

P = 128          # SBUF partitions
QR = 8           # window rows per partition (per image half)
SW = 520         # packed window row stride (elements, mult of 8)
VW_PAD = 514     # window row payload: N + 2 cols
IMG_ROWS = 513   # window rows: N + 1
# per-image element stride in the packed x buffer, 64B aligned
IMG_STRIDE = -(-(IMG_ROWS * SW) // 32) * 32


@dataclass(frozen=True)
class Cfg:
    bpc: int   # images per core (even)
    n: int     # output height/width (512)


def build_nc(cfg: Cfg) -> bass.Bass:
    BPC, N = cfg.bpc, cfg.n
    assert BPC % 2 == 0 and N == 512
    PAIRS = BPC // 2
    VW = N + 2             # 514: vertical-stage strip width
    XR = QR * SW + VW      # elements loaded per partition (8 rows + spill)
    f16 = mybir.dt.float16
    f32 = mybir.dt.float32
    i32 = mybir.dt.int32

    nc = bacc.Bacc("TRN2", target_bir_lowering=False, debug=False)
    x_d = nc.declare_dram_parameter("x", [1, BPC * IMG_STRIDE], f16,
                                    isOutput=False)
    wmat_d = nc.declare_dram_parameter("wmat", [128, PAIRS * 4], f32,
                                       isOutput=False)
    y_d = nc.declare_dram_parameter("y", [BPC, N * N], f16, isOutput=True)

    mult = mybir.AluOpType.mult
    add = mybir.AluOpType.add

    with tile.TileContext(nc) as tc:
        with (
            tc.tile_pool(name="const", bufs=1) as constp,
            tc.tile_pool(name="win", bufs=5) as winp,
            tc.tile_pool(name="hp", bufs=3) as hp,
            tc.tile_pool(name="vp", bufs=3) as vp,
        ):
            # const loads ride the ACT ring so the first window loads on the
            # SP ring aren't queued behind them
            wmat_sb = constp.tile([128, PAIRS * 4], f32, tag="wmat")
            nc.scalar.dma_start(wmat_sb[:], wmat_d[:, :])

            # one zero register: reg+imm offsets keep every DMA on the
            # dynamic (engine-fanned) path with compile-time immediates.
            # The zero comes from an SBUF memset, not a DMA, so the first
            # window load doesn't wait on a const-load round trip.
            zt = constp.tile([1, 1], i32, tag="zero")
            nc.vector.memset(zt[:], 0)
            rz = nc.alloc_register(mybir.EngineType.SP, "rz")
            nc.sync.reg_load(rz, zt[0:1, 0:1])
            svz = nc.snap(rz, donate=True, min_val=0, max_val=0)

            # One load DMA per pair. Fan-out across the 16 DMA engines
            # follows the OUTER AP dim, so the 64 row-groups go outermost;
            # the image index is a middle dim, which interleaves the two
            # images onto even/odd partitions (scalars follow parity).
            # Loads are issued a few pairs ahead of the stores in program
            # order: SP executes its stream in-order, so a store waiting on
            # compute must not head-of-line-block the next prefetch.
            xts = {}

            HALF_A = (QR // 2) * SW + VW  # q 0..4 incl. the q=4 boundary

            def load(t):
                x_t = winp.tile([128, (QR + 1) * SW], f16, tag="x")
                if t == 0:
                    # pair 0's load lands in two pieces so its first
                    # q-half compute starts ~3.5us before the full tile
                    nc.sync.dma_start(
                        x_t[:, 0:HALF_A],
                        bass.AP(x_d, svz + 0,
                                [[QR * SW, 64], [IMG_STRIDE, 2],
                                 [1, HALF_A]]),
                    )
                    nc.sync.dma_start(
                        x_t[:, HALF_A:XR],
                        bass.AP(x_d, svz + HALF_A,
                                [[QR * SW, 64], [IMG_STRIDE, 2],
                                 [1, XR - HALF_A]]),
                    )
                else:
                    nc.sync.dma_start(
                        x_t[:, 0:XR],
                        bass.AP(x_d, svz + (2 * t) * IMG_STRIDE,
                                [[QR * SW, 64], [IMG_STRIDE, 2],
                                 [XR // 2, 2], [1, XR // 2]]),
                    )
                xts[t] = x_t

            PRE = 4
            for t in range(min(PRE, PAIRS)):
                load(t)
            for t in range(PAIRS):
                x_t = xts.pop(t)
                x3 = x_t[:].rearrange("p (q w) -> p q w", w=SW)

                wy0 = wmat_sb[:, 4 * t + 0: 4 * t + 1]
                wy1 = wmat_sb[:, 4 * t + 1: 4 * t + 2]
                wx0 = wmat_sb[:, 4 * t + 2: 4 * t + 3]
                wx1 = wmat_sb[:, 4 * t + 3: 4 * t + 4]

                # vertical blend on VW-wide strips
                mv1 = hp.tile([128, QR * VW], f16, tag="mv1")
                mv2 = hp.tile([128, QR * VW], f16, tag="mv2")
                vv = hp.tile([128, QR * VW], f16, tag="vv")
                mv13 = mv1[:].rearrange("p (q w) -> p q w", w=VW)
                mv23 = mv2[:].rearrange("p (q w) -> p q w", w=VW)
                vv3 = vv[:].rearrange("p (q w) -> p q w", w=VW)
                # both vertical muls on ACT: they depend only on the input
                # window, so ACT streams ahead while DVE owns the rest of
                # the chain (no ACT<->DVE ping-pong within a pair). For
                # pair 0 only, mv1 runs on DVE in parallel with ACT's mv2,
                # pulling the whole DVE stream (the end-critical path)
                # ~4us earlier.
                if t == 0:
                    # pair 0 computes its vertical stage in q-halves so it
                    # can start on the first half-load
                    Hq = QR // 2
                    for h in range(2):
                        q0, q1 = h * Hq, h * Hq + Hq
                        nc.scalar.mul(mv23[:, q0:q1, :],
                                      x3[:, q0 + 1:q1 + 1, 0:VW], wy1)
                        nc.vector.tensor_scalar(
                            mv13[:, q0:q1, :], x3[:, q0:q1, 0:VW], wy0,
                            None, op0=mult)
                elif t <= 2:
                    nc.scalar.mul(mv23[:, :, :], x3[:, 1:QR + 1, 0:VW], wy1)
                    nc.vector.tensor_scalar(
                        mv13[:, :, :], x3[:, 0:QR, 0:VW], wy0, None,
                        op0=mult)
                else:
                    nc.scalar.mul(mv23[:, :, :], x3[:, 1:QR + 1, 0:VW], wy1)
                    nc.scalar.mul(mv13[:, :, :], x3[:, 0:QR, 0:VW], wy0)
                # horizontal blend
                mh1 = vp.tile([128, QR * N], f16, tag="mh1")
                mh2 = vp.tile([128, QR * N], f16, tag="mh2")
                v = vp.tile([128, QR * N], f16, tag="v")
                mh13 = mh1[:].rearrange("p (q w) -> p q w", w=N)
                mh23 = mh2[:].rearrange("p (q w) -> p q w", w=N)

                nc.vector.tensor_tensor(
                    vv3[:, :, :], mv13[:, :, :], mv23[:, :, :], op=add)
                nc.vector.tensor_scalar(
                    mh13[:, :, :], vv3[:, :, 0:N], wx0, None, op0=mult)
                nc.vector.tensor_scalar(
                    mh23[:, :, :], vv3[:, :, 1:N + 1], wx1, None, op0=mult)
                if t + PRE < PAIRS:
                    load(t + PRE)
                # the LAST pair's final add + store drain in q-halves so
                # the first half-store overlaps the second half's add
                nsplit = 2 if t == PAIRS - 1 else 1
                Hs = QR * N // nsplit
                for s in range(nsplit):
                    nc.vector.tensor_tensor(
                        v[:, s * Hs:(s + 1) * Hs], mh1[:, s * Hs:(s + 1) * Hs],
                        mh2[:, s * Hs:(s + 1) * Hs], op=add)
                    nc.sync.dma_start(
                        bass.AP(y_d, svz + (2 * t) * N * N + s * Hs,
                                [[QR * N, 64], [N * N, 2],
                                 [Hs // 2, 2], [1, Hs // 2]]),
                        v[:, s * Hs:(s + 1) * Hs],
                    )
    nc.compile()
    return nc


def host_prep(padded: np.ndarray, positions: np.ndarray, n_cores: int):
    """Shard + build metadata. padded: (B, npad, npad) f32, positions: (B, 2)."""
    B, npad, _ = padded.shape
    n = npad - 10
    bpc = B // n_cores

    px = positions[:, 0].astype(np.float32)
    py = positions[:, 1].astype(np.float32)
    fy = np.floor(py)
    fx = np.floor(px)
    ay = (5 + fy).astype(np.int64)
    ax = (5 + fx).astype(np.int64)
    wy = (py - fy).astype(np.float32)
    wx = (px - fx).astype(np.float32)

    # zero margins so every (513, 514) window is in bounds
    m_lo = int(max(0, -min(ay.min(), ax.min())))
    m_hi = int(max(0, max(ay.max(), ax.max()) + VW_PAD - npad))
    wpad = npad + m_lo + m_hi

    pp = np.zeros((B, wpad, wpad), dtype=np.float16)
    pp[:, m_lo:m_lo + npad, m_lo:m_lo + npad] = padded.astype(np.float16)
    A = ay + m_lo
    Bc = ax + m_lo

    cfg = Cfg(bpc=bpc, n=n)

    in_maps = []
    for cidx in range(n_cores):
        xbuf = np.zeros((bpc, IMG_STRIDE), dtype=np.float16)
        wmat = np.empty((bpc // 2, 128, 4), dtype=np.float32)
        for j in range(bpc):
            g = cidx * bpc + j
            w = pp[g, A[g]:A[g] + IMG_ROWS, Bc[g]:Bc[g] + VW_PAD]
            xbuf[j, :IMG_ROWS * SW].reshape(IMG_ROWS, SW)[:, :VW_PAD] = w
            # images of a pair are interleaved on even/odd partitions
            wmat[j // 2, j % 2::2, 0] = 1.0 - wy[g]
            wmat[j // 2, j % 2::2, 1] = wy[g]
            wmat[j // 2, j % 2::2, 2] = 1.0 - wx[g]
            wmat[j // 2, j % 2::2, 3] = wx[g]
        in_maps.append({
            "x": xbuf.reshape(1, -1),
            # kernel reads wmat as [128 partitions, PAIRS*4]
            "wmat": np.ascontiguousarray(
                wmat.transpose(1, 0, 2).reshape(128, -1)),
        })
    return cfg, in_maps


N_CORES = 8
_nc_cache: dict = {}


def kernel(padded_obj: np.ndarray, positions: np.ndarray) -> np.ndarray:
    padded_obj = np.asarray(padded_obj)
    positions = np.asarray(positions)
    B, npad, _, C = padded_obj.shape
    cfg, in_maps = host_prep(
        padded_obj.reshape(B, npad, npad).astype(np.float32, copy=False),
        positions, N_CORES)

    nc = _nc_cache.get(cfg)
    if nc is None:
        nc = build_nc(cfg)
        _nc_cache[cfg] = nc

    res = run_bass_kernel_spmd(nc, in_maps, core_ids=list(range(N_CORES)))
    out = np.concatenate([r["y"] for r in res.results], axis=0)
    return out.reshape(B, cfg.n, cfg.n, 1).astype(np.float32)
